# revision 5
# baseline (speedup 1.0000x reference)
"""CostVolumeLayer Trainium2 kernel.

Computes the local cost volume: for search_range R=4,
  out[b, di*9+dj, i, j] = sum_c src[b,c,i,j] * tgt_zp[b,c,i-2R+di, j-2R+dj]
(tgt zero-padded outside its bounds; the window is OFF-CENTER, covering
tgt rows i-8..i and cols j-8..j - faithful to the torch reference, whose
window indices index the zero-padded tensor directly and whose negative
indices wrap into the zero pad).

Strategy (8 NeuronCores, SPMD):
  - Shard: core c -> batch b = c//2, W-half wh = c%2 (cols 64*wh..64*wh+63).
    Each core gets src shard [C=128, 64, 64] (block-reordered) and a
    zero-padded tgt halo shard [C=128, 72, 72] in bf16 (host pre-pads the
    interior cols; the device memsets the 8 top halo rows).
  - Device: for each 8x16 pixel block, TWO M=64 bf16 matmuls - one per
    pixel-row half (mi in 0..3 / 4..7) - each streaming only its 12x24=288
    window band, both writing the same [128, 288] PSUM bank at partition
    offsets 0/64 (bass auto-derives PE col-strip tile_position from the
    output base partition).  One full-128-partition fp32->fp16 copy per
    block moves the band to SBUF (alternating DVE/ACT), and GRP blocks
    leave as one plain DMA.
  - Host: zero-FLOP banded-diagonal gather from the band blocks into the
    [B, 81, H, W] output (the 81 needed entries per pixel live at
    n = ((mi%4)+di)*24 + (mj+dj), a per-partition-skewed pattern that
    engine access patterns cannot express on-chip).
"""

import numpy as np

R = 4
D = 2 * R + 1          # 9
B, C, H, W = 4, 128, 64, 128
NCORES = 8
WS = W // 2            # 64 cols per core shard
TH = H + R + 4         # 72 padded tgt rows (halo R rows + R zero on top only)
TW = WS + 2 * R        # 72 padded tgt cols
BI, BJ = 8, 16         # pixel block: 8 rows x 16 cols = 128 = M
NBI, NBJ = H // BI, WS // BJ   # 8 x 4 = 32 blocks per core
HALF = 4               # pixel rows per matmul half (M = HALF*BJ = 64)
WIN_I = HALF + 2 * R   # 12 window rows per half
WIN_J = BJ + 2 * R     # 24 window cols
BANDW = WIN_I * WIN_J  # 288 streamed columns per half-matmul
NBLK = NBI * NBJ       # 32
GRP = 4                # blocks per output DMA group
NGRP = NBLK // GRP     # 8 output DMAs

_compiled = None


def _build_bass():
    import concourse.mybir as mybir
    from concourse import bacc
    from concourse.tile import TileContext
    from concourse.tile_rust import add_dep_helper

    f32 = mybir.dt.float32
    bf16 = mybir.dt.bfloat16
    fp16 = mybir.dt.float16
    nc = bacc.Bacc()
    # single combined input: [C, src block-reordered (64*64) ++ tgt payload
    # rows 8..72 of the padded [72, 72] shard (64*72)]
    SRCE = H * WS                  # 4096
    TPAY = (TH - 8) * TW           # 4608 payload cols (rows 8..72)
    E = SRCE + TPAY
    inp = nc.dram_tensor("inp", [C, E], bf16, kind="ExternalInput")
    gout = nc.dram_tensor("gout", [NGRP, 128, GRP * BANDW], fp16,
                          kind="ExternalOutput")
    gout_ap = gout.ap()

    with TileContext(nc) as tc:
        with (
            tc.tile_pool(name="inp", bufs=1) as inp_pool,
            tc.tile_pool(name="g", bufs=NGRP) as gpool,
            tc.tile_pool(name="psum", bufs=7, space="PSUM") as psum_pool,
            tc.tile_pool(name="warmpsum", bufs=1, space="PSUM") as warm_pool,
        ):
            # src arrives block-reordered from the host: [C, blk, 128 pixels]
            # so each block's weights are one contiguous free dim.  tgt tile
            # holds the full padded [72, 72] shard; only rows 8..72 are
            # DMA-filled, rows 0..8 are memset (the top zero halo).
            a = inp_pool.tile([C, SRCE + TH * TW], bf16)
            s_v = a[:, :SRCE]
            t_v = a[:, SRCE:].rearrange("c (i j) -> c i j", j=TW)

            # top halo rows 0..8: zero once before any matmul reads them
            nc.vector.memset(t_v[:, 0:8, :], 0.0)

            # PE warm-up: dummy matmuls during the input-DMA wait flip the
            # HAM clock gate to 8/8 before the real matmuls start (needs
            # ~3.5us of sustained PE busy; 9 cold N=512 matmuls ~= 3.8us).
            warm = inp_pool.tile([128, 512], bf16)
            nc.vector.memset(warm, 0.0)
            wps = warm_pool.tile([1, 512], f32)
            for _ in range(9):
                nc.tensor.matmul(wps, warm[:, :1], warm, start=True, stop=True)

            # Chunked input load: src in 8 block-row chunks (512 cols),
            # tgt payload in 8 row chunks (8 rows = 576 cols).  Linear
            # dependency chain so the earliest chunks get full bandwidth
            # (block-row bi needs S_bi and T_0..T_bi).
            iv = inp.ap()
            chunks = []
            for i in range(8):
                so = i * NBJ * 128
                to = SRCE + i * 8 * TW
                chunks.append(nc.sync.dma_start(
                    out=a[:, so:so + NBJ * 128], in_=iv[:, so:so + NBJ * 128]))
                chunks.append(nc.sync.dma_start(
                    out=a[:, 8 * TW + to:8 * TW + to + 8 * TW],
                    in_=iv[:, to:to + 8 * TW]))
            for i in range(1, len(chunks)):
                add_dep_helper(chunks[i].ins, chunks[i - 1].ins,
                               reason="input chunks drain in pipeline order")

            for grp in range(NGRP):
                stage = gpool.tile([128, GRP * BANDW], fp16)
                for k in range(GRP):
                    blk = grp * GRP + k
                    bi, bj = divmod(blk, NBJ)
                    # full-bank tile so pool buffers never share a PSUM bank
                    psraw = psum_pool.tile([128, 512], f32)
                    ps = psraw[:, :BANDW]
                    for h in range(2):
                        lhsT = s_v[:, blk * 128 + 64 * h: blk * 128 + 64 * (h + 1)]
                        rhs = t_v[:, bi * BI + HALF * h: bi * BI + HALF * h + WIN_I,
                                  bj * BJ: bj * BJ + WIN_J]
                        nc.tensor.matmul(ps[64 * h:64 * (h + 1), :], lhsT, rhs,
                                         start=True, stop=True)
                    dst = stage[:, k * BANDW:(k + 1) * BANDW]
                    if blk % 2 == 0:
                        nc.vector.tensor_copy(dst, ps)
                    else:
                        nc.scalar.copy(dst, ps)
                nc.sync.dma_start(out=gout_ap[grp], in_=stage)
    nc.finalize()
    return nc


def _get_compiled():
    global _compiled
    if _compiled is None:
        _compiled = _build_bass()
    return _compiled


def _shard_inputs(src, tgt):
    """Build per-core input maps (host-side shard + zero-pad + bf16)."""
    import ml_dtypes

    bf16 = ml_dtypes.bfloat16
    in_maps = []
    for c in range(NCORES):
        b = c // 2
        w0 = WS * (c % 2)
        # block-reorder: [C, NBI, BI, NBJ, BJ] -> [C, (NBI NBJ), (BI BJ)]
        s = np.ascontiguousarray(
            src[b, :, :, w0:w0 + WS]
            .reshape(C, NBI, BI, NBJ, BJ)
            .transpose(0, 1, 3, 2, 4)
        ).reshape(C, H * WS)
        # tgt payload: padded rows 8..72 of the [72, 72] shard.  Padded
        # (q, x) holds tgt (q - 8, w0 + x - 8); the window for output pixel
        # (i, j_local) covers padded rows i..i+8, cols j_local..j_local+8
        # = tgt rows i-8..i, cols w0+j_local-8..w0+j_local (the off-center
        # reference window).
        tp = np.zeros((C, TH - 8, TW), dtype=np.float32)
        clo = max(w0 - 8, 0)
        chi = min(w0 + WS, W)
        tp[:, :, clo - (w0 - 8): clo - (w0 - 8) + (chi - clo)] = \
            tgt[b, :, :, clo:chi]
        inp = np.concatenate([s, tp.reshape(C, (TH - 8) * TW)], axis=1)
        in_maps.append({"inp": np.ascontiguousarray(inp.astype(bf16))})
    return in_maps


# host-side gather indices: out[k=(di,dj)] at pixel (mi,mj) of a block sits
# in half h = mi // 4 at band column n = ((mi%4)+di)*WIN_J + (mj+dj)
_mi = np.arange(BI)[:, None, None, None]
_mj = np.arange(BJ)[None, :, None, None]
_di = np.arange(D)[None, None, :, None]
_dj = np.arange(D)[None, None, None, :]
_NIDX = (((_mi % HALF) + _di) * WIN_J + (_mj + _dj)).reshape(BI, BJ, D * D)


def _unshard_output(results):
    out = np.empty((B, D * D, H, W), dtype=np.float32)
    for c in range(NCORES):
        b = c // 2
        w0 = WS * (c % 2)
        g = (results[c]["gout"]
             .astype(np.float32)
             .reshape(NGRP, 128, GRP, BANDW)
             .transpose(0, 2, 1, 3)
             .reshape(NBI, NBJ, BI, BJ, BANDW))
        # gather: v[bi,bj,mi,mj,k] = g[bi,bj,mi,mj,_NIDX[mi,mj,k]]
        v = np.take_along_axis(g, _NIDX[None, None], axis=-1)
        # -> out[b, k, bi*8+mi, w0+bj*16+mj]
        v = v.transpose(4, 0, 2, 1, 3)  # [81, NBI, BI, NBJ, BJ]
        out[b, :, :, w0:w0 + WS] = v.reshape(D * D, H, WS)
    return out


def kernel(src, tgt):
    from concourse.bass_utils import run_bass_kernel_spmd

    src = np.asarray(src, dtype=np.float32)
    tgt = np.asarray(tgt, dtype=np.float32)
    nc = _get_compiled()
    in_maps = _shard_inputs(src, tgt)
    res = run_bass_kernel_spmd(nc, in_maps, core_ids=list(range(NCORES)))
    return _unshard_output(res.results)


# revision 6
# speedup vs baseline: 2.1699x; 2.1699x over previous
"""CostVolumeLayer Trainium2 kernel.

Computes the local cost volume: for search_range R=4,
  out[b, di*9+dj, i, j] = sum_c src[b,c,i,j] * tgt_zp[b,c,i-2R+di, j-2R+dj]
(tgt zero-padded outside its bounds; the window is OFF-CENTER, covering
tgt rows i-8..i and cols j-8..j - faithful to the torch reference, whose
window indices index the zero-padded tensor directly and whose negative
indices wrap into the zero pad).

Strategy (8 NeuronCores, SPMD):
  - Shard: core c -> batch b = c//2, W-half wh = c%2 (cols 64*wh..64*wh+63).
    Each core gets src shard [C=128, 64, 64] (block-reordered) and a
    zero-padded tgt halo shard [C=128, 72, 72] in bf16 (host pre-pads the
    interior cols; the device memsets the 8 top halo rows).
  - Device: for each 8x16 pixel block, TWO M=64 bf16 matmuls - one per
    pixel-row half (mi in 0..3 / 4..7) - each streaming only its 12x24=288
    window band, both writing the same [128, 288] PSUM bank at partition
    offsets 0/64 (bass auto-derives PE col-strip tile_position from the
    output base partition).  One full-128-partition fp32->fp16 copy per
    block moves the band to SBUF (alternating DVE/ACT), and GRP blocks
    leave as one plain DMA.
  - Host: zero-FLOP banded-diagonal gather from the band blocks into the
    [B, 81, H, W] output (the 81 needed entries per pixel live at
    n = ((mi%4)+di)*24 + (mj+dj), a per-partition-skewed pattern that
    engine access patterns cannot express on-chip).
"""

import numpy as np

R = 4
D = 2 * R + 1          # 9
B, C, H, W = 4, 128, 64, 128
NCORES = 8
WS = W // 2            # 64 cols per core shard
TH = H + R + 4         # 72 padded tgt rows (halo R rows + R zero on top only)
TW = WS + 2 * R        # 72 padded tgt cols
BI, BJ = 8, 16         # pixel block: 8 rows x 16 cols = 128 = M
NBI, NBJ = H // BI, WS // BJ   # 8 x 4 = 32 blocks per core
HALF = 4               # pixel rows per matmul half (M = HALF*BJ = 64)
WIN_I = HALF + 2 * R   # 12 window rows per half
WIN_J = BJ + 2 * R     # 24 window cols
BANDW = WIN_I * WIN_J  # 288 streamed columns per half-matmul
NBLK = NBI * NBJ       # 32
GRP = 4                # blocks per output DMA group
NGRP = NBLK // GRP     # 8 output DMAs

_compiled = None


def _build_bass():
    import concourse.mybir as mybir
    from concourse import bacc
    from concourse.tile import TileContext
    from concourse.tile_rust import add_dep_helper

    f32 = mybir.dt.float32
    bf16 = mybir.dt.bfloat16
    fp16 = mybir.dt.float16
    nc = bacc.Bacc()
    # single combined input: [C, src block-reordered (64*64) ++ tgt payload
    # rows 8..72 of the padded [72, 72] shard (64*72)]
    SRCE = H * WS                  # 4096
    TPAY = (TH - 8) * TW           # 4608 payload cols (rows 8..72)
    E = SRCE + TPAY
    inp = nc.dram_tensor("inp", [C, E], bf16, kind="ExternalInput")
    gout = nc.dram_tensor("gout", [NGRP, 128, GRP * BANDW], fp16,
                          kind="ExternalOutput")
    gout_ap = gout.ap()

    with TileContext(nc) as tc:
        with (
            tc.tile_pool(name="inp", bufs=1) as inp_pool,
            tc.tile_pool(name="g", bufs=NGRP) as gpool,
            tc.tile_pool(name="psum", bufs=7, space="PSUM") as psum_pool,
            tc.tile_pool(name="warmpsum", bufs=1, space="PSUM") as warm_pool,
        ):
            # src arrives block-reordered from the host: [C, blk, 128 pixels]
            # so each block's weights are one contiguous free dim.  tgt tile
            # holds the full padded [72, 72] shard; only rows 8..72 are
            # DMA-filled, rows 0..8 are memset (the top zero halo).
            a = inp_pool.tile([C, SRCE + TH * TW], bf16)
            s_v = a[:, :SRCE]
            t_v = a[:, SRCE:].rearrange("c (i j) -> c i j", j=TW)

            # top halo rows 0..8: zero once before any matmul reads them
            nc.vector.memset(t_v[:, 0:8, :], 0.0)

            # PE warm-up: dummy matmuls during the input-DMA wait flip the
            # HAM clock gate to 8/8 before the real matmuls start (needs
            # ~3.5us of sustained PE busy; 9 cold N=512 matmuls ~= 3.8us).
            warm = inp_pool.tile([128, 512], bf16)
            nc.vector.memset(warm, 0.0)
            wps = warm_pool.tile([1, 512], f32)
            for _ in range(9):
                nc.tensor.matmul(wps, warm[:, :1], warm, start=True, stop=True)

            # Chunked input load: src in 8 block-row chunks (512 cols),
            # tgt payload in 8 row chunks (8 rows = 576 cols), issued in
            # consumption order S0,T0,S1,T1,...  NO dependency chain: DMAs
            # on one HWDGE ring drain FIFO in issue order at full bandwidth
            # and complete incrementally; chaining them instead inserts a
            # ~2us sem-wait + descriptor-regen gap per chunk (measured).
            iv = inp.ap()
            for i in range(8):
                so = i * NBJ * 128
                to = SRCE + i * 8 * TW
                nc.sync.dma_start(
                    out=a[:, so:so + NBJ * 128], in_=iv[:, so:so + NBJ * 128])
                nc.sync.dma_start(
                    out=a[:, 8 * TW + to:8 * TW + to + 8 * TW],
                    in_=iv[:, to:to + 8 * TW])

            for grp in range(NGRP):
                stage = gpool.tile([128, GRP * BANDW], fp16)
                for k in range(GRP):
                    blk = grp * GRP + k
                    bi, bj = divmod(blk, NBJ)
                    # full-bank tile so pool buffers never share a PSUM bank
                    psraw = psum_pool.tile([128, 512], f32)
                    ps = psraw[:, :BANDW]
                    for h in range(2):
                        lhsT = s_v[:, blk * 128 + 64 * h: blk * 128 + 64 * (h + 1)]
                        rhs = t_v[:, bi * BI + HALF * h: bi * BI + HALF * h + WIN_I,
                                  bj * BJ: bj * BJ + WIN_J]
                        nc.tensor.matmul(ps[64 * h:64 * (h + 1), :], lhsT, rhs,
                                         start=True, stop=True)
                    dst = stage[:, k * BANDW:(k + 1) * BANDW]
                    if blk % 2 == 0:
                        nc.vector.tensor_copy(dst, ps)
                    else:
                        nc.scalar.copy(dst, ps)
                nc.sync.dma_start(out=gout_ap[grp], in_=stage)
    nc.finalize()
    return nc


def _get_compiled():
    global _compiled
    if _compiled is None:
        _compiled = _build_bass()
    return _compiled


def _shard_inputs(src, tgt):
    """Build per-core input maps (host-side shard + zero-pad + bf16)."""
    import ml_dtypes

    bf16 = ml_dtypes.bfloat16
    in_maps = []
    for c in range(NCORES):
        b = c // 2
        w0 = WS * (c % 2)
        # block-reorder: [C, NBI, BI, NBJ, BJ] -> [C, (NBI NBJ), (BI BJ)]
        s = np.ascontiguousarray(
            src[b, :, :, w0:w0 + WS]
            .reshape(C, NBI, BI, NBJ, BJ)
            .transpose(0, 1, 3, 2, 4)
        ).reshape(C, H * WS)
        # tgt payload: padded rows 8..72 of the [72, 72] shard.  Padded
        # (q, x) holds tgt (q - 8, w0 + x - 8); the window for output pixel
        # (i, j_local) covers padded rows i..i+8, cols j_local..j_local+8
        # = tgt rows i-8..i, cols w0+j_local-8..w0+j_local (the off-center
        # reference window).
        tp = np.zeros((C, TH - 8, TW), dtype=np.float32)
        clo = max(w0 - 8, 0)
        chi = min(w0 + WS, W)
        tp[:, :, clo - (w0 - 8): clo - (w0 - 8) + (chi - clo)] = \
            tgt[b, :, :, clo:chi]
        inp = np.concatenate([s, tp.reshape(C, (TH - 8) * TW)], axis=1)
        in_maps.append({"inp": np.ascontiguousarray(inp.astype(bf16))})
    return in_maps


# host-side gather indices: out[k=(di,dj)] at pixel (mi,mj) of a block sits
# in half h = mi // 4 at band column n = ((mi%4)+di)*WIN_J + (mj+dj)
_mi = np.arange(BI)[:, None, None, None]
_mj = np.arange(BJ)[None, :, None, None]
_di = np.arange(D)[None, None, :, None]
_dj = np.arange(D)[None, None, None, :]
_NIDX = (((_mi % HALF) + _di) * WIN_J + (_mj + _dj)).reshape(BI, BJ, D * D)


def _unshard_output(results):
    out = np.empty((B, D * D, H, W), dtype=np.float32)
    for c in range(NCORES):
        b = c // 2
        w0 = WS * (c % 2)
        g = (results[c]["gout"]
             .astype(np.float32)
             .reshape(NGRP, 128, GRP, BANDW)
             .transpose(0, 2, 1, 3)
             .reshape(NBI, NBJ, BI, BJ, BANDW))
        # gather: v[bi,bj,mi,mj,k] = g[bi,bj,mi,mj,_NIDX[mi,mj,k]]
        v = np.take_along_axis(g, _NIDX[None, None], axis=-1)
        # -> out[b, k, bi*8+mi, w0+bj*16+mj]
        v = v.transpose(4, 0, 2, 1, 3)  # [81, NBI, BI, NBJ, BJ]
        out[b, :, :, w0:w0 + WS] = v.reshape(D * D, H, WS)
    return out


def kernel(src, tgt):
    from concourse.bass_utils import run_bass_kernel_spmd

    src = np.asarray(src, dtype=np.float32)
    tgt = np.asarray(tgt, dtype=np.float32)
    nc = _get_compiled()
    in_maps = _shard_inputs(src, tgt)
    res = run_bass_kernel_spmd(nc, in_maps, core_ids=list(range(NCORES)))
    return _unshard_output(res.results)
